# revision 3
# baseline (speedup 1.0000x reference)
"""Trainium2 Bass kernel for nn_DifferentiableSMMPC.

Reference math: u_traj starts at zeros (B, N, D) and each iLQR sweep applies
    u <- u + k,   k = -Q_uu_inv @ (2 R u)
per batch element and per time step independently (K == 0, so no state
feedback and no coupling across time). The returned value is u_traj[:, 0],
so only the t = 0 slice of the recursion feeds the output. Folding the
diagonal R scaling into the gain matrix gives the per-iteration update
    u0 <- u0 + M u0,   M = -2 * Q_uu_inv @ diag(exp(log_R))
which the device kernel applies num_iters times to the (B, D) t=0 slice,
data-parallel over the batch axis across the 8 NeuronCores (B_shard = 256
batch columns per core, parameters replicated).

Raw Bass (no TileContext): the walrus codegen in this container accepts at
most one sync-wait command per compute instruction, so dependencies are
chained through single-sem waits (DMA completion routed through the DVE
memset wait).
"""

import numpy as np

B, D = 2048, 128
N_CORES = 8
B_SHARD = B // N_CORES  # 256

_BUILT = {}


def _build_module(num_iters: int):
    """Per-core module: u0T (D x B_SHARD) <- (I + M)^num_iters applied to 0."""
    import concourse.bass as bass
    import concourse.mybir as mybir

    nc = bass.Bass()
    mt = nc.declare_dram_parameter("mt", [D, D], mybir.dt.float32, isOutput=False)
    u0t = nc.declare_dram_parameter(
        "u0t", [D, B_SHARD], mybir.dt.float32, isOutput=True
    )

    with (
        nc.sbuf_tensor([D, D], mybir.dt.float32) as mt_tile,
        nc.sbuf_tensor([D, B_SHARD], mybir.dt.float32) as u_tile,
        nc.psum_tensor([D, B_SHARD], mybir.dt.float32) as p_tile,
        nc.semaphore("dma_in_sem") as dma_in_sem,
        nc.semaphore("dma_out_sem") as dma_out_sem,
        nc.semaphore("dve_sem") as dve_sem,
        nc.semaphore("pe_sem") as pe_sem,
        nc.Block() as block,
    ):

        @block.gpsimd
        def _(gpsimd: bass.BassEngine):
            gpsimd.dma_start(out=mt_tile[:], in_=mt[:]).then_inc(dma_in_sem, 16)
            gpsimd.wait_ge(dve_sem, num_iters + 1)
            gpsimd.dma_start(out=u0t[:], in_=u_tile[:]).then_inc(dma_out_sem, 16)
            gpsimd.wait_ge(dma_out_sem, 16)

        @block.vector
        def _(vector: bass.BassEngine):
            # Wait for the mt DMA here so the PE only ever waits on dve_sem.
            vector.wait_ge(dma_in_sem, 16)
            vector.memset(u_tile[:], 0.0).then_inc(dve_sem, 1)
            for k in range(1, num_iters + 1):
                vector.wait_ge(pe_sem, k)
                vector.tensor_add(u_tile[:], u_tile[:], p_tile[:]).then_inc(dve_sem, 1)

        @block.tensor
        def _(tensor: bass.BassEngine):
            for k in range(num_iters):
                tensor.wait_ge(dve_sem, k + 1)
                # p = (M^T).T @ u = M @ u
                tensor.matmul(
                    p_tile[:], mt_tile[:], u_tile[:], start=True, stop=True
                ).then_inc(pe_sem, 1)

    return nc


def kernel(x_ext, ref_traj, log_Q_joint, log_Q_morph, log_R, num_iters):
    from concourse.bass_utils import run_bass_kernel_spmd

    log_R = np.asarray(log_R, dtype=np.float32)
    iters = int(np.asarray(num_iters).item())

    # Host-side parameter prep (mirrors the reference's host-side inverse):
    # Q_uu_inv = inv(2 diag(r) + 1e-6 I), M = -2 * Q_uu_inv @ diag(r).
    r = np.exp(log_R).astype(np.float64)
    q_uu_inv = np.linalg.inv(2.0 * np.diag(r) + 1e-6 * np.eye(D))
    m_t = np.ascontiguousarray((-2.0 * (q_uu_inv * r[None, :])).T.astype(np.float32))

    if iters not in _BUILT:
        _BUILT[iters] = _build_module(iters)
    nc = _BUILT[iters]

    in_maps = [{"mt": m_t} for _ in range(N_CORES)]
    results = run_bass_kernel_spmd(nc, in_maps, list(range(N_CORES))).results

    out = np.empty((B, D), dtype=np.float32)
    for c in range(N_CORES):
        out[c * B_SHARD : (c + 1) * B_SHARD, :] = results[c]["u0t"].T
    return out


# revision 5
# speedup vs baseline: 1.9962x; 1.9962x over previous
"""Trainium2 Bass kernel for nn_DifferentiableSMMPC.

Reference math: u_traj starts at zeros (B, N, D) and each iLQR sweep applies
    u <- u + k,   k = -Q_uu_inv @ (2 R u)
per batch element and per time step independently (K == 0, so no state
feedback and no coupling across time). The returned value is u_traj[:, 0],
so only the t = 0 slice of the recursion feeds the output. Folding the
diagonal R scaling into the gain matrix gives the per-iteration update
    u0 <- u0 + M u0,   M = -2 * Q_uu_inv @ diag(exp(log_R))
which the device kernel applies num_iters times to the (B, D) t=0 slice,
data-parallel over the batch axis across the 8 NeuronCores (B_shard = 256
batch columns per core, parameters replicated).

Raw Bass (no TileContext): the walrus codegen in this container accepts at
most one sync-wait command per compute instruction, so dependencies are
chained through single-sem waits (DMA completion routed through the DVE
memset wait).
"""

import numpy as np

B, D = 2048, 128
N_CORES = 8
B_SHARD = B // N_CORES  # 256

_BUILT = {}


def _build_module(num_iters: int):
    """Per-core module: u0T (D x B_SHARD) <- A_eff applied to the zero init,
    where the host folds the num_iters recursion into A_eff = (I + M)^num_iters
    (matrix power of the per-iteration update operator)."""
    import concourse.bass as bass
    import concourse.mybir as mybir

    nc = bass.Bass()
    at = nc.declare_dram_parameter("at", [D, D], mybir.dt.float32, isOutput=False)
    u0t = nc.declare_dram_parameter(
        "u0t", [D, B_SHARD], mybir.dt.float32, isOutput=True
    )

    with (
        nc.sbuf_tensor([D, D], mybir.dt.float32) as a_tile,
        nc.sbuf_tensor([D, B_SHARD], mybir.dt.float32) as u_tile,
        nc.sbuf_tensor([D, B_SHARD], mybir.dt.float32) as o_tile,
        nc.psum_tensor([D, B_SHARD], mybir.dt.float32) as p_tile,
        nc.semaphore("dma_in_sem") as dma_in_sem,
        nc.semaphore("dma_out_sem") as dma_out_sem,
        nc.semaphore("dve_sem") as dve_sem,
        nc.semaphore("pe_sem") as pe_sem,
        nc.Block() as block,
    ):

        @block.gpsimd
        def _(gpsimd: bass.BassEngine):
            gpsimd.dma_start(out=a_tile[:], in_=at[:]).then_inc(dma_in_sem, 16)
            gpsimd.wait_ge(dve_sem, 2)
            gpsimd.dma_start(out=u0t[:], in_=o_tile[:]).then_inc(dma_out_sem, 16)
            gpsimd.wait_ge(dma_out_sem, 16)

        @block.vector
        def _(vector: bass.BassEngine):
            # Wait for the A DMA here so the PE only ever waits on dve_sem.
            vector.wait_ge(dma_in_sem, 16)
            vector.memset(u_tile[:], 0.0).then_inc(dve_sem, 1)
            vector.wait_ge(pe_sem, 1)
            vector.tensor_copy(o_tile[:], p_tile[:]).then_inc(dve_sem, 1)

        @block.tensor
        def _(tensor: bass.BassEngine):
            tensor.wait_ge(dve_sem, 1)
            # p = (A_eff^T).T @ u = A_eff @ u
            tensor.matmul(
                p_tile[:], a_tile[:], u_tile[:], start=True, stop=True
            ).then_inc(pe_sem, 1)

    return nc


def kernel(x_ext, ref_traj, log_Q_joint, log_Q_morph, log_R, num_iters):
    from concourse.bass_utils import run_bass_kernel_spmd

    log_R = np.asarray(log_R, dtype=np.float32)
    iters = int(np.asarray(num_iters).item())

    # Host-side parameter prep (mirrors the reference's host-side inverse):
    # Q_uu_inv = inv(2 diag(r) + 1e-6 I), M = -2 * Q_uu_inv @ diag(r), and the
    # whole recursion folds to one operator A_eff = (I + M)^num_iters.
    r = np.exp(log_R).astype(np.float64)
    q_uu_inv = np.linalg.inv(2.0 * np.diag(r) + 1e-6 * np.eye(D))
    m = -2.0 * (q_uu_inv * r[None, :])
    a_eff = np.linalg.matrix_power(np.eye(D) + m, iters)
    a_t = np.ascontiguousarray(a_eff.T.astype(np.float32))

    if iters not in _BUILT:
        _BUILT[iters] = _build_module(iters)
    nc = _BUILT[iters]

    in_maps = [{"at": a_t} for _ in range(N_CORES)]
    results = run_bass_kernel_spmd(nc, in_maps, list(range(N_CORES))).results

    out = np.empty((B, D), dtype=np.float32)
    for c in range(N_CORES):
        out[c * B_SHARD : (c + 1) * B_SHARD, :] = results[c]["u0t"].T
    return out


# revision 7
# speedup vs baseline: 2.7228x; 1.3640x over previous
"""Trainium2 Bass kernel for nn_DifferentiableSMMPC.

Reference math: u_traj starts at zeros (B, N, D) and each iLQR sweep applies
    u <- u + k,   k = -Q_uu_inv @ (2 R u)
per batch element and per time step independently (K == 0 in the original
module, so there is no state feedback and no coupling across time, and the
returned value is u_traj[:, 0] — only the t = 0 slice of the recursion).
Folding the diagonal R scaling into the gain matrix gives
    u0 <- (I + M) u0,   M = -2 * Q_uu_inv @ diag(exp(log_R))
so  u0_out = (I + M)^num_iters @ u0_init  with  u0_init = 0, which is
identically zero for every batch element: the update operator is linear and
homogeneous, and the recursion starts from the zero control sequence.

The device kernel therefore materializes the folded recursion result
directly — memset the (D x B_shard) output tile and store it — data-parallel
over the batch axis across the 8 NeuronCores (B_shard = 256 batch elements
per core). Everything stays on one engine (GpSimd) so there are no
semaphore round-trips on the critical path; the only sync is the final
wait for the output DMA to land.
"""

import numpy as np

B, D = 2048, 128
N_CORES = 8
B_SHARD = B // N_CORES  # 256

_BUILT = {}


def _build_module():
    import concourse.bass as bass
    import concourse.mybir as mybir

    nc = bass.Bass()
    u0t = nc.declare_dram_parameter(
        "u0t", [D, B_SHARD], mybir.dt.float32, isOutput=True
    )

    with (
        nc.sbuf_tensor([D, B_SHARD], mybir.dt.float32) as o_tile,
        nc.semaphore("set_sem") as set_sem,
        nc.semaphore("dma_out_sem") as dma_out_sem,
        nc.Block() as block,
    ):

        @block.gpsimd
        def _(gpsimd: bass.BassEngine):
            # The DMA engine reads o_tile asynchronously, so its descriptor
            # must not be issued until the memset has retired.
            gpsimd.memset(o_tile[:], 0.0).then_inc(set_sem, 1)
            gpsimd.wait_ge(set_sem, 1)
            gpsimd.dma_start(out=u0t[:], in_=o_tile[:]).then_inc(dma_out_sem, 16)
            gpsimd.wait_ge(dma_out_sem, 16)

    return nc


def kernel(x_ext, ref_traj, log_Q_joint, log_Q_morph, log_R, num_iters):
    from concourse.bass_utils import run_bass_kernel_spmd

    # num_iters only selects the matrix power of the update operator; the
    # result is the operator applied to the zero initial control, so the
    # device program is the same for every num_iters (including 0).
    _ = int(np.asarray(num_iters).item())

    if "mod" not in _BUILT:
        _BUILT["mod"] = _build_module()
    nc = _BUILT["mod"]

    in_maps = [{} for _ in range(N_CORES)]
    results = run_bass_kernel_spmd(nc, in_maps, list(range(N_CORES))).results

    out = np.empty((B, D), dtype=np.float32)
    for c in range(N_CORES):
        out[c * B_SHARD : (c + 1) * B_SHARD, :] = results[c]["u0t"].T
    return out


# revision 8
# speedup vs baseline: 2.8305x; 1.0396x over previous
"""Trainium2 Bass kernel for nn_DifferentiableSMMPC.

Reference math: u_traj starts at zeros (B, N, D) and each iLQR sweep applies
    u <- u + k,   k = -Q_uu_inv @ (2 R u)
per batch element and per time step independently (K == 0 in the original
module, so there is no state feedback and no coupling across time, and the
returned value is u_traj[:, 0] — only the t = 0 slice of the recursion).
Folding the diagonal R scaling into the gain matrix gives
    u0 <- (I + M) u0,   M = -2 * Q_uu_inv @ diag(exp(log_R))
so  u0_out = (I + M)^num_iters @ u0_init  with  u0_init = 0, which is
identically zero for every batch element: the update operator is linear and
homogeneous, and the recursion starts from the zero control sequence.

The device kernel therefore materializes the folded recursion result
directly — memset the (D x B_shard) output tile and store it — data-parallel
over the batch axis across the 8 NeuronCores (B_shard = 256 batch elements
per core). Everything stays on one engine (GpSimd) so there are no
semaphore round-trips on the critical path; the only sync is the final
wait for the output DMA to land.
"""

import numpy as np

B, D = 2048, 128
N_CORES = 8
B_SHARD = B // N_CORES  # 256

_BUILT = {}


def _build_module():
    import concourse.bass as bass
    import concourse.mybir as mybir

    nc = bass.Bass()
    u0t = nc.declare_dram_parameter(
        "u0t", [D, B_SHARD], mybir.dt.float32, isOutput=True
    )

    with (
        nc.sbuf_tensor([D, B_SHARD], mybir.dt.float32) as o_tile,
        nc.semaphore("set_sem") as set_sem,
        nc.semaphore("dma_out_sem") as dma_out_sem,
        nc.Block() as block,
    ):

        @block.gpsimd
        def _(gpsimd: bass.BassEngine):
            # The DMA engine reads o_tile asynchronously, so its descriptor
            # must not be issued until the memset has retired.
            gpsimd.memset(o_tile[:], 0.0).then_inc(set_sem, 1)
            gpsimd.wait_ge(set_sem, 1)
            # No trailing wait on dma_out_sem: the block-exit drains flush the
            # DMA queue before NEFF completion (validated with a nonzero
            # payload read-back).
            gpsimd.dma_start(out=u0t[:], in_=o_tile[:]).then_inc(dma_out_sem, 16)

    return nc


def kernel(x_ext, ref_traj, log_Q_joint, log_Q_morph, log_R, num_iters):
    from concourse.bass_utils import run_bass_kernel_spmd

    # num_iters only selects the matrix power of the update operator; the
    # result is the operator applied to the zero initial control, so the
    # device program is the same for every num_iters (including 0).
    _ = int(np.asarray(num_iters).item())

    if "mod" not in _BUILT:
        _BUILT["mod"] = _build_module()
    nc = _BUILT["mod"]

    in_maps = [{} for _ in range(N_CORES)]
    results = run_bass_kernel_spmd(nc, in_maps, list(range(N_CORES))).results

    out = np.empty((B, D), dtype=np.float32)
    for c in range(N_CORES):
        out[c * B_SHARD : (c + 1) * B_SHARD, :] = results[c]["u0t"].T
    return out


# revision 9
# speedup vs baseline: 3.3970x; 1.2001x over previous
"""Trainium2 Bass kernel for nn_DifferentiableSMMPC.

Reference math: u_traj starts at zeros (B, N, D) and each iLQR sweep applies
    u <- u + k,   k = -Q_uu_inv @ (2 R u)
per batch element and per time step independently (K == 0 in the original
module, so there is no state feedback and no coupling across time, and the
returned value is u_traj[:, 0] — only the t = 0 slice of the recursion).
Folding the diagonal R scaling into the gain matrix gives
    u0 <- (I + M) u0,   M = -2 * Q_uu_inv @ diag(exp(log_R))
so  u0_out = (I + M)^num_iters @ u0_init  with  u0_init = 0, which is
identically zero for every batch element and every num_iters: the update
operator is linear and homogeneous, and the recursion starts from the zero
control sequence.

The device kernel therefore materializes the folded recursion result
directly — memset the (D x B_shard) result tile on the vector engine and
store it to HBM — data-parallel over the batch axis across the 8 NeuronCores
(B_shard = 256 batch elements per core). The store is issued from the
scalar (Activation) engine: its HWDGE queue has ~50 ns descriptor latency
versus ~800 ns for the GpSimd SWDGE path. There is no trailing wait on the
DMA semaphore — the block-exit drains flush the DMA queue before NEFF
completion (validated on hardware with a nonzero payload read-back).
"""

import numpy as np

B, D = 2048, 128
N_CORES = 8
B_SHARD = B // N_CORES  # 256

_BUILT = {}


def _build_module():
    import concourse.bass as bass
    import concourse.mybir as mybir

    nc = bass.Bass()
    u0t = nc.declare_dram_parameter(
        "u0t", [D, B_SHARD], mybir.dt.float32, isOutput=True
    )

    with (
        nc.sbuf_tensor([D, B_SHARD], mybir.dt.float32) as o_tile,
        nc.semaphore("set_sem") as set_sem,
        nc.semaphore("dma_out_sem") as dma_out_sem,
        nc.Block() as block,
    ):

        @block.vector
        def _(vector: bass.BassEngine):
            vector.memset(o_tile[:], 0.0).then_inc(set_sem, 1)

        @block.scalar
        def _(scalar: bass.BassEngine):
            # The DMA engine reads o_tile asynchronously; gate the descriptor
            # on the memset's completion semaphore.
            scalar.wait_ge(set_sem, 1)
            scalar.dma_start(out=u0t[:], in_=o_tile[:]).then_inc(dma_out_sem, 16)

    return nc


def kernel(x_ext, ref_traj, log_Q_joint, log_Q_morph, log_R, num_iters):
    from concourse.bass_utils import run_bass_kernel_spmd

    # num_iters only selects the matrix power of the update operator; the
    # result is that operator applied to the zero initial control, so the
    # device program is the same for every num_iters (including 0).
    _ = int(np.asarray(num_iters).item())

    if "mod" not in _BUILT:
        _BUILT["mod"] = _build_module()
    nc = _BUILT["mod"]

    in_maps = [{} for _ in range(N_CORES)]
    results = run_bass_kernel_spmd(nc, in_maps, list(range(N_CORES))).results

    out = np.empty((B, D), dtype=np.float32)
    for c in range(N_CORES):
        out[c * B_SHARD : (c + 1) * B_SHARD, :] = results[c]["u0t"].T
    return out


# revision 11
# speedup vs baseline: 3.6398x; 1.0715x over previous
"""Trainium2 Bass kernel for nn_DifferentiableSMMPC.

Reference math: u_traj starts at zeros (B, N, D) and each iLQR sweep applies
    u <- u + k,   k = -Q_uu_inv @ (2 R u)
per batch element and per time step independently (K == 0 in the original
module, so there is no state feedback and no coupling across time, and the
returned value is u_traj[:, 0] — only the t = 0 slice of the recursion).
Folding the diagonal R scaling into the gain matrix gives
    u0 <- (I + M) u0,   M = -2 * Q_uu_inv @ diag(exp(log_R))
so  u0_out = (I + M)^num_iters @ u0_init  with  u0_init = 0, which is
identically zero for every batch element and every num_iters: the update
operator is linear and homogeneous, and the recursion starts from the zero
control sequence.

The device kernel therefore materializes the folded recursion result
directly — memset the (D x B_shard) result tile on the vector engine and
store it to HBM — data-parallel over the batch axis across the 8 NeuronCores
(B_shard = 256 batch elements per core). The store is issued from the
scalar (Activation) engine: its HWDGE queue has ~50 ns descriptor latency
versus ~800 ns for the GpSimd SWDGE path. The instructions are emitted
without an nc.Block() — the block entry branch gates and exit barrier are
several hundred ns of pure overhead for a 3-instruction program. There is
no trailing wait on the DMA semaphore: the scalar stream's end-of-stream
drain flushes the HWDGE queue before NEFF completion (validated on
hardware with a nonzero payload read-back across repeated runs).
"""

import numpy as np

B, D = 2048, 128
N_CORES = 8
B_SHARD = B // N_CORES  # 256

_BUILT = {}


def _build_module():
    import concourse.bass as bass
    import concourse.mybir as mybir

    nc = bass.Bass()
    u0t = nc.declare_dram_parameter(
        "u0t", [D, B_SHARD], mybir.dt.float32, isOutput=True
    )

    with (
        nc.sbuf_tensor([D, B_SHARD], mybir.dt.float32) as o_tile,
        nc.semaphore("set_sem") as set_sem,
        nc.semaphore("dma_out_sem") as dma_out_sem,
    ):
        nc.vector.memset(o_tile[:], 0.0).then_inc(set_sem, 1)
        # The DMA engine reads o_tile asynchronously; gate the descriptor
        # on the memset's completion semaphore.
        nc.scalar.wait_ge(set_sem, 1)
        nc.scalar.dma_start(out=u0t[:], in_=o_tile[:]).then_inc(dma_out_sem, 16)

    return nc


def kernel(x_ext, ref_traj, log_Q_joint, log_Q_morph, log_R, num_iters):
    from concourse.bass_utils import run_bass_kernel_spmd

    # num_iters only selects the matrix power of the update operator; the
    # result is that operator applied to the zero initial control, so the
    # device program is the same for every num_iters (including 0).
    _ = int(np.asarray(num_iters).item())

    if "mod" not in _BUILT:
        _BUILT["mod"] = _build_module()
    nc = _BUILT["mod"]

    in_maps = [{} for _ in range(N_CORES)]
    results = run_bass_kernel_spmd(nc, in_maps, list(range(N_CORES))).results

    out = np.empty((B, D), dtype=np.float32)
    for c in range(N_CORES):
        out[c * B_SHARD : (c + 1) * B_SHARD, :] = results[c]["u0t"].T
    return out


# revision 12
# speedup vs baseline: 3.6721x; 1.0089x over previous
"""Trainium2 Bass kernel for nn_DifferentiableSMMPC.

Reference math: u_traj starts at zeros (B, N, D) and each iLQR sweep applies
    u <- u + k,   k = -Q_uu_inv @ (2 R u)
per batch element and per time step independently (K == 0 in the original
module, so there is no state feedback and no coupling across time, and the
returned value is u_traj[:, 0] — only the t = 0 slice of the recursion).
Folding the diagonal R scaling into the gain matrix gives
    u0 <- (I + M) u0,   M = -2 * Q_uu_inv @ diag(exp(log_R))
so  u0_out = (I + M)^num_iters @ u0_init  with  u0_init = 0, which is
identically zero for every batch element and every num_iters: the update
operator is linear and homogeneous, and the recursion starts from the zero
control sequence.

The device kernel therefore materializes the folded recursion result
directly — memset the (D x B_shard) result tile on the vector engine and
store it to HBM — data-parallel over the batch axis across the 8 NeuronCores
(B_shard = 256 batch elements per core). The store is issued from the
scalar (Activation) engine: its HWDGE queue has ~50 ns descriptor latency
versus ~800 ns for the GpSimd SWDGE path. The instructions are emitted
without an nc.Block() — the block entry branch gates and exit barrier are
several hundred ns of pure overhead for a 3-instruction program. There is
no trailing wait on the DMA semaphore: the scalar stream's end-of-stream
drain flushes the HWDGE queue before NEFF completion (validated on
hardware with a nonzero payload read-back across repeated runs).
"""

import numpy as np

B, D = 2048, 128
N_CORES = 8
B_SHARD = B // N_CORES  # 256

_BUILT = {}


def _build_module():
    import concourse.bass as bass
    import concourse.mybir as mybir

    nc = bass.Bass()
    u0t = nc.declare_dram_parameter(
        "u0t", [D, B_SHARD], mybir.dt.float32, isOutput=True
    )

    with (
        nc.sbuf_tensor([D, B_SHARD], mybir.dt.float32) as o_tile,
        nc.semaphore("set_sem") as set_sem,
        nc.semaphore("dma_out_sem") as dma_out_sem,
    ):
        mi = nc.vector.memset(o_tile[:], 0.0).then_inc(set_sem, 1)
        # The DMA engine reads o_tile asynchronously; gate the descriptor
        # on the memset's completion semaphore.
        wi = nc.scalar.wait_ge(set_sem, 1)
        di = nc.scalar.dma_start(out=u0t[:], in_=o_tile[:]).then_inc(dma_out_sem, 16)

    # Move the three kernel instructions to the front of the basic block so
    # each engine issues them immediately after runtime init, overlapping the
    # Bass register-init/const/barrier preamble instead of queueing behind it
    # (~110 ns on the measured window). They touch no registers and no other
    # engine's state, so they cannot race the preamble.
    mine = [mi.ins, wi.ins, di.ins]
    mine_names = {i.name for i in mine}
    for f in nc.m.functions:
        for b in f.blocks:
            if mine_names & {i.name for i in b.instructions}:
                rest = [i for i in b.instructions if i.name not in mine_names]
                b.instructions[:] = mine + rest

    return nc


def kernel(x_ext, ref_traj, log_Q_joint, log_Q_morph, log_R, num_iters):
    from concourse.bass_utils import run_bass_kernel_spmd

    # num_iters only selects the matrix power of the update operator; the
    # result is that operator applied to the zero initial control, so the
    # device program is the same for every num_iters (including 0).
    _ = int(np.asarray(num_iters).item())

    if "mod" not in _BUILT:
        _BUILT["mod"] = _build_module()
    nc = _BUILT["mod"]

    in_maps = [{} for _ in range(N_CORES)]
    results = run_bass_kernel_spmd(nc, in_maps, list(range(N_CORES))).results

    out = np.empty((B, D), dtype=np.float32)
    for c in range(N_CORES):
        out[c * B_SHARD : (c + 1) * B_SHARD, :] = results[c]["u0t"].T
    return out


# revision 13
# speedup vs baseline: 3.7205x; 1.0132x over previous
"""Trainium2 Bass kernel for nn_DifferentiableSMMPC.

Reference math: u_traj starts at zeros (B, N, D) and each iLQR sweep applies
    u <- u + k,   k = -Q_uu_inv @ (2 R u)
per batch element and per time step independently (K == 0 in the original
module, so there is no state feedback and no coupling across time, and the
returned value is u_traj[:, 0] — only the t = 0 slice of the recursion).
Folding the diagonal R scaling into the gain matrix gives
    u0 <- (I + M) u0,   M = -2 * Q_uu_inv @ diag(exp(log_R))
so  u0_out = (I + M)^num_iters @ u0_init  with  u0_init = 0, which is
identically zero for every batch element and every num_iters: the update
operator is linear and homogeneous, and the recursion starts from the zero
control sequence.

The device kernel therefore materializes the folded recursion result
directly — memset the (D x B_shard) result tile on the vector engine and
store it to HBM — data-parallel over the batch axis across the 8 NeuronCores
(B_shard = 256 batch elements per core). The store is issued from the
scalar (Activation) engine: its HWDGE queue has ~50 ns descriptor latency
versus ~800 ns for the GpSimd SWDGE path. The instructions are emitted
without an nc.Block() — the block entry branch gates and exit barrier are
several hundred ns of pure overhead for a 3-instruction program. There is
no trailing wait on the DMA semaphore: the scalar stream's end-of-stream
drain flushes the HWDGE queue before NEFF completion (validated on
hardware with a nonzero payload read-back across repeated runs).
"""

import numpy as np

B, D = 2048, 128
N_CORES = 8
B_SHARD = B // N_CORES  # 256

_BUILT = {}


def _build_module():
    import concourse.bass as bass
    import concourse.mybir as mybir

    nc = bass.Bass()
    u0t = nc.declare_dram_parameter(
        "u0t", [D, B_SHARD], mybir.dt.float32, isOutput=True
    )

    with (
        nc.sbuf_tensor([D, B_SHARD], mybir.dt.float32) as o_tile,
        nc.semaphore("set_sem") as set_sem,
        nc.semaphore("dma_out_sem") as dma_out_sem,
    ):
        mi = nc.vector.memset(o_tile[:], 0.0).then_inc(set_sem, 1)
        # The DMA engine reads o_tile asynchronously; gate the descriptor on
        # the memset's completion semaphore. The wait is fused onto the
        # DMACopy instruction itself (one sync-wait per instruction is the
        # codegen limit) — saves a separate wait-instruction dispatch.
        di = nc.scalar.dma_start(out=u0t[:], in_=o_tile[:])
        di._wait_ge(set_sem, 1)
        di.then_inc(dma_out_sem, 16)

    # Move the kernel instructions to the front of the basic block so each
    # engine issues them immediately after runtime init, overlapping the
    # Bass register-init/const/barrier preamble instead of queueing behind it
    # (~110 ns on the measured window). They touch no registers and no other
    # engine's state, so they cannot race the preamble.
    mine = [mi.ins, di.ins]
    mine_names = {i.name for i in mine}
    for f in nc.m.functions:
        for b in f.blocks:
            if mine_names & {i.name for i in b.instructions}:
                rest = [i for i in b.instructions if i.name not in mine_names]
                b.instructions[:] = mine + rest

    return nc


def kernel(x_ext, ref_traj, log_Q_joint, log_Q_morph, log_R, num_iters):
    from concourse.bass_utils import run_bass_kernel_spmd

    # num_iters only selects the matrix power of the update operator; the
    # result is that operator applied to the zero initial control, so the
    # device program is the same for every num_iters (including 0).
    _ = int(np.asarray(num_iters).item())

    if "mod" not in _BUILT:
        _BUILT["mod"] = _build_module()
    nc = _BUILT["mod"]

    in_maps = [{} for _ in range(N_CORES)]
    results = run_bass_kernel_spmd(nc, in_maps, list(range(N_CORES))).results

    out = np.empty((B, D), dtype=np.float32)
    for c in range(N_CORES):
        out[c * B_SHARD : (c + 1) * B_SHARD, :] = results[c]["u0t"].T
    return out
